# revision 5
# baseline (speedup 1.0000x reference)
"""Additive (Bahdanau) attention on 8 Trainium2 NeuronCores.

Reference math (BS=2, J=512, T=256, D=512):
    kk = k @ Wk.T                  [b, J, D]
    qq = q @ Wq.T + bq             [b, T, D]
    scores[b,j,t] = sum_d we[d] * tanh(kk[b,j,d] + qq[b,t,d])
    scores masked to -1e9 where mask[b,j,0]==0
    alphas = softmax_j(scores^T)   [b, T, J]
    context = alphas @ v           [b, T, D]
    returns (context, alphas)

Sharding: the 512 (b, t) query rows are split into 8 blocks of 64 (cores 0-3
take b=0, cores 4-7 take b=1); softmax over j is independent per row.

Sparsity: masked j rows produce exactly-zero alphas (exp(-1e9-max) underflows),
so the host compacts k/v to the unmasked j set before launch and scatters
alphas back afterwards.  This halves the dominant tanh work.

Device pipeline per core (Jp = padded compact J):
    1. PE: kkT[e, j] (stays in PSUM, 4 banks) and qqT[e, t] projections,
       with bq added via a rank-1 matmul (bq outer ones).
    2. Per (t, e-chunk): ACT computes tanh(kkT_chunk + qq[:, t]) using the
       activation unit's per-partition bias port; PE then reduces over e
       with a stationary that is `we_chunk` embedded in column t of a
       sliding window, accumulating every t's scores row into one PSUM
       bank (rows != t accumulate exact zeros).  A rank-1 matmul adds
       -1e9 to the padding columns.
    3. Row softmax: DVE -max, ACT exp (fused bias=-max, fused row-sum
       accumulator), DVE reciprocal + scale.
    4. PE transposes alphas (identity matmul), PE context matmul, DMA out.
"""

import sys

sys.path.insert(0, "/opt/trn_rl_repo")

import numpy as np
from contextlib import ExitStack

import concourse.bass as bass
import concourse.bacc as bacc
import concourse.tile as tile
from concourse import mybir
from concourse.bass_utils import run_bass_kernel_spmd

BS, J, T, D = 2, 512, 256, 512
NCORES = 8
TBLK = BS * T // NCORES  # 64 query rows per core
EC = D // 128            # 4 feature chunks
F32 = mybir.dt.float32
AF = mybir.ActivationFunctionType

_BUILD_CACHE: dict[int, bass.Bass] = {}


def build_nc(jp: int) -> bass.Bass:
    """Build the single-core Bass program (SPMD across all 8 cores)."""
    nc = bacc.Bacc("TRN2", target_bir_lowering=False, debug=True)

    kT = nc.dram_tensor("kT", [D, jp], F32, kind="ExternalInput")
    qT = nc.dram_tensor("qT", [D, TBLK], F32, kind="ExternalInput")
    vc = nc.dram_tensor("vc", [jp, D], F32, kind="ExternalInput")
    WkT = nc.dram_tensor("WkT", [D, D], F32, kind="ExternalInput")
    WqT = nc.dram_tensor("WqT", [D, D], F32, kind="ExternalInput")
    bqv = nc.dram_tensor("bqv", [1, D], F32, kind="ExternalInput")
    wew = nc.dram_tensor("wew", [EC, 128, 255], F32, kind="ExternalInput")
    mrow = nc.dram_tensor("mrow", [1, jp], F32, kind="ExternalInput")
    iden = nc.dram_tensor("iden", [TBLK, TBLK], F32, kind="ExternalInput")
    ctx_out = nc.dram_tensor("ctx_out", [TBLK, D], F32, kind="ExternalOutput")
    alp_out = nc.dram_tensor("alp_out", [TBLK, jp], F32, kind="ExternalOutput")

    jch = [(i * 128, min(128, jp - i * 128)) for i in range((jp + 127) // 128)]

    with tile.TileContext(nc) as tc, ExitStack() as ctx:
        const = ctx.enter_context(tc.tile_pool(name="const", bufs=1))
        work = ctx.enter_context(tc.tile_pool(name="work", bufs=2))
        tanhp = ctx.enter_context(tc.tile_pool(name="tanhp", bufs=8))
        pkk = ctx.enter_context(tc.tile_pool(name="pkk", bufs=4, space="PSUM"))
        pqq = ctx.enter_context(tc.tile_pool(name="pqq", bufs=1, space="PSUM"))
        psc = ctx.enter_context(tc.tile_pool(name="psc", bufs=1, space="PSUM"))
        ptr = ctx.enter_context(tc.tile_pool(name="ptr", bufs=1, space="PSUM"))
        pcx = ctx.enter_context(tc.tile_pool(name="pcx", bufs=1, space="PSUM"))

        # ---------------- loads ----------------
        sb_kT = const.tile([128, EC, jp], F32, tag="kT")
        nc.sync.dma_start(out=sb_kT, in_=kT[:, :].rearrange("(c p) j -> p c j", p=128))
        sb_qT = const.tile([128, EC, TBLK], F32, tag="qT")
        nc.sync.dma_start(out=sb_qT, in_=qT[:, :].rearrange("(c p) t -> p c t", p=128))
        sb_Wk = const.tile([128, EC, D], F32, tag="Wk")
        nc.sync.dma_start(out=sb_Wk, in_=WkT[:, :].rearrange("(c p) e -> p c e", p=128))
        sb_Wq = const.tile([128, EC, D], F32, tag="Wq")
        nc.sync.dma_start(out=sb_Wq, in_=WqT[:, :].rearrange("(c p) e -> p c e", p=128))
        sb_wew = const.tile([128, EC, 255], F32, tag="wew")
        nc.sync.dma_start(out=sb_wew, in_=wew[:, :, :].rearrange("c p w -> p c w"))
        sb_v = []
        for i, (j0, jw) in enumerate(jch):
            t_ = const.tile([jw, D], F32, tag=f"v{i}")
            nc.sync.dma_start(out=t_, in_=vc[j0 : j0 + jw, :])
            sb_v.append(t_)
        sb_mrow = const.tile([1, jp], F32, tag="mrow")
        nc.sync.dma_start(out=sb_mrow, in_=mrow[:, :])
        sb_bq = const.tile([1, D], F32, tag="bq")
        nc.sync.dma_start(out=sb_bq, in_=bqv[:, :])
        sb_id = const.tile([TBLK, TBLK], F32, tag="iden")
        nc.sync.dma_start(out=sb_id, in_=iden[:, :])
        on1 = const.tile([1, 128], F32, tag="on1")
        nc.vector.memset(on1, 1.0)
        on64 = const.tile([1, TBLK], F32, tag="on64")
        nc.vector.memset(on64, 1.0)

        # ---------------- projections ----------------
        # kkT[e-chunk] = sum_c WkT[c-chunk, e-chunk].T @ kT[c-chunk]  (PSUM-resident)
        kk_ps = []
        for e in range(EC):
            kt = pkk.tile([128, jp], F32, tag="kk")
            for c in range(EC):
                nc.tensor.matmul(
                    out=kt,
                    lhsT=sb_Wk[:, c, e * 128 : (e + 1) * 128],
                    rhs=sb_kT[:, c, :],
                    start=(c == 0),
                    stop=(c == EC - 1),
                )
            kk_ps.append(kt)
        # qqT[e-chunk] = sum_c WqT[c,e].T @ qT[c] + bq[e] x ones
        sb_qq = []
        for e in range(EC):
            qps = pqq.tile([128, TBLK], F32, tag="qq")
            for c in range(EC):
                nc.tensor.matmul(
                    out=qps,
                    lhsT=sb_Wq[:, c, e * 128 : (e + 1) * 128],
                    rhs=sb_qT[:, c, :],
                    start=(c == 0),
                    stop=False,
                )
            nc.tensor.matmul(
                out=qps,
                lhsT=sb_bq[0:1, e * 128 : (e + 1) * 128],
                rhs=on64,
                start=False,
                stop=True,
            )
            qs = const.tile([128, TBLK], F32, tag=f"qq{e}")
            nc.vector.tensor_copy(qs, qps)
            sb_qq.append(qs)

        # ---------------- energy + scores ----------------
        scores_ps = psc.tile([128, jp], F32, tag="scores")
        # pad columns get -1e9 in every row: rank-1 ones^T x mrow
        nc.tensor.matmul(
            out=scores_ps, lhsT=on1, rhs=sb_mrow, start=True, stop=False
        )
        for t in range(TBLK):
            for c in range(EC):
                th = tanhp.tile([128, jp], F32, tag="th")
                nc.scalar.activation(
                    out=th,
                    in_=kk_ps[c][:, :],
                    func=AF.Tanh,
                    bias=sb_qq[c][:, t : t + 1],
                    scale=1.0,
                )
                nc.tensor.matmul(
                    out=scores_ps,
                    lhsT=sb_wew[:, c, 127 - t : 255 - t],
                    rhs=th,
                    start=False,
                    stop=(t == TBLK - 1 and c == EC - 1),
                )

        # ---------------- softmax over j ----------------
        neg_max = work.tile([TBLK, 1], F32, tag="negmax")
        nc.vector.tensor_reduce(
            out=neg_max,
            in_=scores_ps[0:TBLK, :],
            axis=mybir.AxisListType.X,
            op=mybir.AluOpType.max,
            negate=True,
        )
        expt = work.tile([TBLK, jp], F32, tag="expt")
        row_sum = work.tile([TBLK, 1], F32, tag="rowsum")
        nc.scalar.activation(
            out=expt,
            in_=scores_ps[0:TBLK, :],
            func=AF.Exp,
            bias=neg_max,
            scale=1.0,
            accum_out=row_sum,
        )
        rinv = work.tile([TBLK, 1], F32, tag="rinv")
        nc.vector.reciprocal(rinv, row_sum)
        alphas = work.tile([TBLK, jp], F32, tag="alphas")
        nc.vector.tensor_scalar_mul(alphas, expt, rinv)
        nc.sync.dma_start(out=alp_out[:, :], in_=alphas)

        # ---------------- context = alphas @ v ----------------
        ctx_ps = pcx.tile([TBLK, D], F32, tag="ctx")
        for i, (j0, jw) in enumerate(jch):
            tr = ptr.tile([jw, TBLK], F32, tag="tr")
            nc.tensor.transpose(tr, alphas[:, j0 : j0 + jw], sb_id)
            alpT = work.tile([jw, TBLK], F32, tag="alpT")
            nc.vector.tensor_copy(alpT, tr)
            nc.tensor.matmul(
                out=ctx_ps,
                lhsT=alpT,
                rhs=sb_v[i],
                start=(i == 0),
                stop=(i == len(jch) - 1),
            )
        ctx_sb = work.tile([TBLK, D], F32, tag="ctxsb")
        nc.vector.tensor_copy(ctx_sb, ctx_ps)
        nc.sync.dma_start(out=ctx_out[:, :], in_=ctx_sb)

    # The axon/PJRT execution path serializes the module without calling
    # finalize(), but Bacc's compile passes (reg alloc, wait splitting)
    # must run before lowering.
    nc.finalize()
    return nc


def _prep(k, v, q, mask, Wq, bq, Wk, we):
    """Host-side layout prep: mask compaction, transposes, padding."""
    idx = [np.flatnonzero(mask[b, :, 0] != 0) for b in range(BS)]
    ju = [len(ix) for ix in idx]
    jmax = max(max(ju), 1)
    jp = ((jmax + 31) // 32) * 32
    kTc, vcs, mrows = [], [], []
    for b in range(BS):
        kt = np.zeros((D, jp), np.float32)
        kt[:, : ju[b]] = k[b][idx[b]].T
        kTc.append(kt)
        vv = np.zeros((jp, D), np.float32)
        vv[: ju[b]] = v[b][idx[b]]
        vcs.append(vv)
        mr = np.zeros((1, jp), np.float32)
        mr[0, ju[b] :] = -1e9
        mrows.append(mr)
    wewin = np.zeros((EC, 128, 255), np.float32)
    for c in range(EC):
        wewin[c, :, 127] = we[c * 128 : (c + 1) * 128]
    common = {
        "WkT": np.ascontiguousarray(Wk.T),
        "WqT": np.ascontiguousarray(Wq.T),
        "bqv": bq.reshape(1, D).astype(np.float32),
        "wew": wewin,
        "iden": np.eye(TBLK, dtype=np.float32),
    }
    qTb = [np.ascontiguousarray(q[b].T) for b in range(BS)]
    in_maps = []
    for core in range(NCORES):
        b = core // (NCORES // BS)
        t0 = (core % (NCORES // BS)) * TBLK
        in_maps.append(
            {
                "kT": kTc[b],
                "qT": np.ascontiguousarray(qTb[b][:, t0 : t0 + TBLK]),
                "vc": vcs[b],
                "mrow": mrows[b],
                **common,
            }
        )
    return in_maps, idx, ju, jp


def kernel(**inputs):
    k = np.asarray(inputs["k"], np.float32)
    v = np.asarray(inputs["v"], np.float32)
    q = np.asarray(inputs["q"], np.float32)
    mask = np.asarray(inputs["mask"])
    Wq = np.asarray(inputs["Wq"], np.float32)
    bq = np.asarray(inputs["bq"], np.float32)
    Wk = np.asarray(inputs["Wk"], np.float32)
    we = np.asarray(inputs["we"], np.float32)

    in_maps, idx, ju, jp = _prep(k, v, q, mask, Wq, bq, Wk, we)
    if jp not in _BUILD_CACHE:
        _BUILD_CACHE[jp] = build_nc(jp)
    nc = _BUILD_CACHE[jp]
    res = run_bass_kernel_spmd(nc, in_maps, core_ids=list(range(NCORES))).results

    context = np.zeros((BS, T, D), np.float32)
    alphas = np.zeros((BS, T, J), np.float32)
    for core in range(NCORES):
        b = core // (NCORES // BS)
        t0 = (core % (NCORES // BS)) * TBLK
        context[b, t0 : t0 + TBLK] = res[core]["ctx_out"]
        alphas[b, t0 : t0 + TBLK, idx[b]] = res[core]["alp_out"][:, : ju[b]].T
    # Degenerate all-masked batch (cannot occur for random masks): reference
    # softmax of an all -1e9 row is uniform.
    for b in range(BS):
        if ju[b] == 0:
            alphas[b] = 1.0 / J
            context[b] = alphas[b] @ v[b]
    return context, alphas


# revision 7
# speedup vs baseline: 1.7097x; 1.7097x over previous
"""Additive (Bahdanau) attention on 8 Trainium2 NeuronCores.

Reference math (BS=2, J=512, T=256, D=512):
    kk = k @ Wk.T                  [b, J, D]
    qq = q @ Wq.T + bq             [b, T, D]
    scores[b,j,t] = sum_d we[d] * tanh(kk[b,j,d] + qq[b,t,d])
    scores masked to -1e9 where mask[b,j,0]==0
    alphas = softmax_j(scores^T)   [b, T, J]
    context = alphas @ v           [b, T, D]
    returns (context, alphas)

Sharding: the 512 (b, t) query rows are split into 8 blocks of 64 (cores 0-3
take b=0, cores 4-7 take b=1); softmax over j is independent per row.

Sparsity: masked j rows produce exactly-zero alphas (exp(-1e9-max) underflows),
so the host compacts k/v to the unmasked j set before launch and scatters
alphas back afterwards.  This halves the dominant tanh work.

Device pipeline per core (jp = padded compact J, bf16 energy path):
    1. PE: kkT[e, j] and qqT[e, t] projections in bf16 (single-pass matmuls),
       bq added via a rank-1 matmul; both evacuated to SBUF as bf16.
    2. Per group of TGRP queries: DVE tensor_scalar_add broadcasts qq[:, t]
       onto kkT (bf16 4x mode) building S supertiles; ACT runs one unbiased
       in-place tanh per (chunk, group) — the big free dim amortizes the
       per-instruction overhead; PE reduces over e with a `we` sliding-window
       stationary (bf16, FWL) that lands each t's scores in its own PSUM row
       of one accumulation group.  A rank-1 fp32 matmul adds -1e9 to pad
       columns.
    3. Row softmax in fp32: DVE -max, ACT exp (bias=-max, fused row-sum),
       DVE reciprocal + scale.
    4. PE transposes alphas (identity matmul), fp32 context matmul, DMA out.
"""

import sys

sys.path.insert(0, "/opt/trn_rl_repo")

import numpy as np
from contextlib import ExitStack

import concourse.bass as bass
import concourse.bacc as bacc
import concourse.tile as tile
from concourse import mybir
from concourse.bass_utils import run_bass_kernel_spmd

BS, J, T, D = 2, 512, 256, 512
NCORES = 8
TBLK = BS * T // NCORES  # 64 query rows per core
EC = D // 128            # 4 feature chunks
TGRP = 8                 # queries per tanh supertile
F32 = mybir.dt.float32
BF16 = mybir.dt.bfloat16
NPBF16 = mybir.dt.np(BF16)
AF = mybir.ActivationFunctionType

_BUILD_CACHE: dict[int, bass.Bass] = {}


def build_nc(jp: int) -> bass.Bass:
    """Build the single-core Bass program (SPMD across all 8 cores)."""
    nc = bacc.Bacc("TRN2", target_bir_lowering=False, debug=True)

    kT = nc.dram_tensor("kT", [D, jp], BF16, kind="ExternalInput")
    qT = nc.dram_tensor("qT", [D, TBLK], BF16, kind="ExternalInput")
    vc = nc.dram_tensor("vc", [jp, D], F32, kind="ExternalInput")
    WkT = nc.dram_tensor("WkT", [D, D], BF16, kind="ExternalInput")
    WqT = nc.dram_tensor("WqT", [D, D], BF16, kind="ExternalInput")
    bqv = nc.dram_tensor("bqv", [1, D], BF16, kind="ExternalInput")
    wew = nc.dram_tensor("wew", [EC, 2, 128, 256], BF16, kind="ExternalInput")
    mrow = nc.dram_tensor("mrow", [1, jp], F32, kind="ExternalInput")
    iden = nc.dram_tensor("iden", [TBLK, TBLK], F32, kind="ExternalInput")
    ctx_out = nc.dram_tensor("ctx_out", [TBLK, D], F32, kind="ExternalOutput")
    alp_out = nc.dram_tensor("alp_out", [TBLK, jp], F32, kind="ExternalOutput")

    jch = [(i * 128, min(128, jp - i * 128)) for i in range((jp + 127) // 128)]

    with tile.TileContext(nc) as tc, ExitStack() as ctx:
        const = ctx.enter_context(tc.tile_pool(name="const", bufs=1))
        work = ctx.enter_context(tc.tile_pool(name="work", bufs=2))
        spool = ctx.enter_context(tc.tile_pool(name="spool", bufs=3))
        pkk = ctx.enter_context(tc.tile_pool(name="pkk", bufs=4, space="PSUM"))
        pqq = ctx.enter_context(tc.tile_pool(name="pqq", bufs=1, space="PSUM"))
        psc = ctx.enter_context(tc.tile_pool(name="psc", bufs=1, space="PSUM"))
        ptr = ctx.enter_context(tc.tile_pool(name="ptr", bufs=1, space="PSUM"))
        pcx = ctx.enter_context(tc.tile_pool(name="pcx", bufs=1, space="PSUM"))

        # ---------------- loads ----------------
        sb_kT = const.tile([128, EC, jp], BF16, tag="kT")
        nc.sync.dma_start(out=sb_kT, in_=kT[:, :].rearrange("(c p) j -> p c j", p=128))
        sb_qT = const.tile([128, EC, TBLK], BF16, tag="qT")
        nc.sync.dma_start(out=sb_qT, in_=qT[:, :].rearrange("(c p) t -> p c t", p=128))
        sb_Wk = const.tile([128, EC, D], BF16, tag="Wk")
        nc.sync.dma_start(out=sb_Wk, in_=WkT[:, :].rearrange("(c p) e -> p c e", p=128))
        sb_Wq = const.tile([128, EC, D], BF16, tag="Wq")
        nc.sync.dma_start(out=sb_Wq, in_=WqT[:, :].rearrange("(c p) e -> p c e", p=128))
        sb_wew = const.tile([128, EC, 2, 256], BF16, tag="wew")
        nc.sync.dma_start(out=sb_wew, in_=wew[:, :, :, :].rearrange("c r p w -> p c r w"))
        sb_v = []
        for i, (j0, jw) in enumerate(jch):
            t_ = const.tile([jw, D], F32, tag=f"v{i}")
            nc.sync.dma_start(out=t_, in_=vc[j0 : j0 + jw, :])
            sb_v.append(t_)
        sb_mrow = const.tile([1, jp], F32, tag="mrow")
        nc.sync.dma_start(out=sb_mrow, in_=mrow[:, :])
        sb_bq = const.tile([1, D], BF16, tag="bq")
        nc.sync.dma_start(out=sb_bq, in_=bqv[:, :])
        sb_id = const.tile([TBLK, TBLK], F32, tag="iden")
        nc.sync.dma_start(out=sb_id, in_=iden[:, :])
        on1 = const.tile([1, 128], F32, tag="on1")
        nc.vector.memset(on1, 1.0)
        on64 = const.tile([1, TBLK], BF16, tag="on64")
        nc.vector.memset(on64, 1.0)

        # ---------------- projections (bf16 in, fp32 PSUM, bf16 out) -------
        kk_sb = const.tile([128, EC, jp], BF16, tag="kksb")
        for e in range(EC):
            kt = pkk.tile([128, jp], F32, tag="kk")
            for c in range(EC):
                nc.tensor.matmul(
                    out=kt,
                    lhsT=sb_Wk[:, c, e * 128 : (e + 1) * 128],
                    rhs=sb_kT[:, c, :],
                    start=(c == 0),
                    stop=(c == EC - 1),
                )
            nc.vector.tensor_copy(kk_sb[:, e, :], kt)
        qq_sb = const.tile([128, EC, TBLK], F32, tag="qqsb")
        for e in range(EC):
            qps = pqq.tile([128, TBLK], F32, tag="qq")
            for c in range(EC):
                nc.tensor.matmul(
                    out=qps,
                    lhsT=sb_Wq[:, c, e * 128 : (e + 1) * 128],
                    rhs=sb_qT[:, c, :],
                    start=(c == 0),
                    stop=False,
                )
            nc.tensor.matmul(
                out=qps,
                lhsT=sb_bq[0:1, e * 128 : (e + 1) * 128],
                rhs=on64,
                start=False,
                stop=True,
            )
            nc.vector.tensor_copy(qq_sb[:, e, :], qps)

        # ---------------- energy + scores ----------------
        scores_ps = psc.tile([128, jp], F32, tag="scores")
        # pad columns get -1e9 in every row: rank-1 ones^T x mrow
        nc.tensor.matmul(
            out=scores_ps, lhsT=on1, rhs=sb_mrow, start=True, stop=False
        )
        ngrp = TBLK // TGRP
        for g in range(ngrp):
            st = spool.tile([128, EC, TGRP, jp], BF16, tag="S")
            for c in range(EC):
                for i in range(TGRP):
                    t = g * TGRP + i
                    nc.vector.tensor_scalar_add(
                        st[:, c, i, :], kk_sb[:, c, :], qq_sb[:, c, t : t + 1]
                    )
            for c in range(EC):
                nc.scalar.activation(
                    out=st[:, c, :, :], in_=st[:, c, :, :], func=AF.Tanh
                )
            for c in range(EC):
                for i in range(TGRP):
                    t = g * TGRP + i
                    par = t & 1
                    o = 128 - t if par == 0 else 127 - t
                    nc.tensor.matmul(
                        out=scores_ps,
                        lhsT=sb_wew[:, c, par, o : o + 128],
                        rhs=st[:, c, i, :],
                        start=False,
                        stop=(g == ngrp - 1 and c == EC - 1 and i == TGRP - 1),
                    )

        # ---------------- softmax over j ----------------
        neg_max = work.tile([TBLK, 1], F32, tag="negmax")
        nc.vector.tensor_reduce(
            out=neg_max,
            in_=scores_ps[0:TBLK, :],
            axis=mybir.AxisListType.X,
            op=mybir.AluOpType.max,
            negate=True,
        )
        expt = work.tile([TBLK, jp], F32, tag="expt")
        row_sum = work.tile([TBLK, 1], F32, tag="rowsum")
        nc.scalar.activation(
            out=expt,
            in_=scores_ps[0:TBLK, :],
            func=AF.Exp,
            bias=neg_max,
            scale=1.0,
            accum_out=row_sum,
        )
        rinv = work.tile([TBLK, 1], F32, tag="rinv")
        nc.vector.reciprocal(rinv, row_sum)
        alphas = work.tile([TBLK, jp], F32, tag="alphas")
        nc.vector.tensor_scalar_mul(alphas, expt, rinv)
        nc.sync.dma_start(out=alp_out[:, :], in_=alphas)

        # ---------------- context = alphas @ v ----------------
        ctx_ps = pcx.tile([TBLK, D], F32, tag="ctx")
        for i, (j0, jw) in enumerate(jch):
            tr = ptr.tile([jw, TBLK], F32, tag="tr")
            nc.tensor.transpose(tr, alphas[:, j0 : j0 + jw], sb_id)
            alpT = work.tile([jw, TBLK], F32, tag="alpT")
            nc.vector.tensor_copy(alpT, tr)
            nc.tensor.matmul(
                out=ctx_ps,
                lhsT=alpT,
                rhs=sb_v[i],
                start=(i == 0),
                stop=(i == len(jch) - 1),
            )
        ctx_sb = work.tile([TBLK, D], F32, tag="ctxsb")
        nc.vector.tensor_copy(ctx_sb, ctx_ps)
        nc.sync.dma_start(out=ctx_out[:, :], in_=ctx_sb)

    # The axon/PJRT execution path serializes the module without calling
    # finalize(), but Bacc's compile passes (reg alloc, wait splitting)
    # must run before lowering.
    nc.finalize()
    return nc


def _prep(k, v, q, mask, Wq, bq, Wk, we):
    """Host-side layout prep: mask compaction, transposes, padding, casts."""
    idx = [np.flatnonzero(mask[b, :, 0] != 0) for b in range(BS)]
    ju = [len(ix) for ix in idx]
    jmax = max(max(ju), 1)
    jp = ((jmax + 7) // 8) * 8
    kTc, vcs, mrows = [], [], []
    for b in range(BS):
        kt = np.zeros((D, jp), NPBF16)
        kt[:, : ju[b]] = k[b][idx[b]].T.astype(NPBF16)
        kTc.append(kt)
        vv = np.zeros((jp, D), np.float32)
        vv[: ju[b]] = v[b][idx[b]]
        vcs.append(vv)
        mr = np.zeros((1, jp), np.float32)
        mr[0, ju[b] :] = -1e9
        mrows.append(mr)
    # we sliding windows: two parity copies so every 128-wide stationary
    # slice starts 4B-aligned in bf16. col t of slice == we iff slice
    # start is (128 - t) in copy 0 (t even) / (127 - t) in copy 1 (t odd).
    wewin = np.zeros((EC, 2, 128, 256), NPBF16)
    for c in range(EC):
        wewin[c, 0, :, 128] = we[c * 128 : (c + 1) * 128].astype(NPBF16)
        wewin[c, 1, :, 127] = we[c * 128 : (c + 1) * 128].astype(NPBF16)
    common = {
        "WkT": np.ascontiguousarray(Wk.T).astype(NPBF16),
        "WqT": np.ascontiguousarray(Wq.T).astype(NPBF16),
        "bqv": bq.reshape(1, D).astype(NPBF16),
        "wew": wewin,
        "iden": np.eye(TBLK, dtype=np.float32),
    }
    qTb = [np.ascontiguousarray(q[b].T).astype(NPBF16) for b in range(BS)]
    in_maps = []
    for core in range(NCORES):
        b = core // (NCORES // BS)
        t0 = (core % (NCORES // BS)) * TBLK
        in_maps.append(
            {
                "kT": kTc[b],
                "qT": np.ascontiguousarray(qTb[b][:, t0 : t0 + TBLK]),
                "vc": vcs[b],
                "mrow": mrows[b],
                **common,
            }
        )
    return in_maps, idx, ju, jp


def kernel(**inputs):
    k = np.asarray(inputs["k"], np.float32)
    v = np.asarray(inputs["v"], np.float32)
    q = np.asarray(inputs["q"], np.float32)
    mask = np.asarray(inputs["mask"])
    Wq = np.asarray(inputs["Wq"], np.float32)
    bq = np.asarray(inputs["bq"], np.float32)
    Wk = np.asarray(inputs["Wk"], np.float32)
    we = np.asarray(inputs["we"], np.float32)

    in_maps, idx, ju, jp = _prep(k, v, q, mask, Wq, bq, Wk, we)
    if jp not in _BUILD_CACHE:
        _BUILD_CACHE[jp] = build_nc(jp)
    nc = _BUILD_CACHE[jp]
    res = run_bass_kernel_spmd(nc, in_maps, core_ids=list(range(NCORES))).results

    context = np.zeros((BS, T, D), np.float32)
    alphas = np.zeros((BS, T, J), np.float32)
    for core in range(NCORES):
        b = core // (NCORES // BS)
        t0 = (core % (NCORES // BS)) * TBLK
        context[b, t0 : t0 + TBLK] = res[core]["ctx_out"]
        alphas[b, t0 : t0 + TBLK, idx[b]] = res[core]["alp_out"][:, : ju[b]].T
    # Degenerate all-masked batch (cannot occur for random masks): reference
    # softmax of an all -1e9 row is uniform.
    for b in range(BS):
        if ju[b] == 0:
            alphas[b] = 1.0 / J
            context[b] = alphas[b] @ v[b]
    return context, alphas
